# revision 20
# baseline (speedup 1.0000x reference)
"""AdaptiveKernelFC Trainium2 kernel (8-core data parallel), v3.

Math: the reference builds per-sample filters w[n,p,c,kh,kw] =
x[n,c,kh,kw]*Wk[p] + bk[p] and convolves x[n] with them (7x7 kernel ==
feature map size, pad 3).  The conv factors exactly:

    y[n,p,i,j] = Wk[p]*S1[n,i,j] + bk[p]*S2[n,i,j] + b_adap[p]

with S1 the 2D autocorrelation of x[n] (summed over channels) and S2
the 7x7 box-filter correlation of the channel sums.  Both come from one
fused matmul pair per sample with stationary [x | ones]: PSUM rows 0:49
hold the spatial Gram matrix G[r, q], rows 49:98 hold the channel sums
xs[q] replicated.

The diagonal band T[p, (n, t)] = row p's value at column (r-24)+t
(r = p mod 49) is produced by a DRAM staging round trip (SBUF DMAs
cannot mix partition and element steps; DRAM is flat): dump rows at
stride 192 starting at column 24, then ONE gather per (region, half)
with row stride 193 -- the +1 slope realizes the per-row shift.  The
staging tensors are inline zero constants, so the out-of-band positions
the gather sweeps through are always finite zeros/neighbor data; a
single mask-multiply (f32 mask broadcast over samples, bf16 output)
zeroes the wrapped positions and casts for the TensorEngine in one op.

The final stage contracts T directly against Q (98, 256) where
Q[p, :] = Wk if p < 49 else bk (built once off the critical path by a
tiny selector matmul), so the selector reduction, bias staging and R
assembly of earlier versions all disappear:

    y[ck*128+p, (n,s)] = (Q[:, ck]^T Tbf)[p, (n,s)] + b_adap
    (b_adap added by the PSUM->SBUF tensor_scalar_add move).

Everything is split into two sample-halves so the second half's Gram
matmuls and round trip overlap the first half's output pipeline.

Sharding: pure data parallel, batch N=32 split 4 samples/core across 8
cores; params replicated; outputs concatenated.
"""

import os
import numpy as np

import concourse.bass as bass
import concourse.bacc as bacc
import concourse.mybir as mybir
import concourse.tile as tile
from concourse.ap import AP
from concourse.bass_utils import run_bass_kernel_spmd

N, C, H, W = 32, 256, 7, 7
P = 256
NCORES = 8
B = N // NCORES          # samples per core
HW = H * W               # 49
ROWS = 2 * HW            # 98 PSUM rows (G region then xs region)
FREE = B * HW            # 196 free columns (n, t)
HF = 2 * HW              # 98 free columns per half
SROW = 192               # staging row stride (f32 elems); band reads stay in-row
F32 = mybir.dt.float32
BF16 = mybir.dt.bfloat16

_cached = {}
last_exec_time_ns = None


def _mask_np():
    # mask[p, t]: r = p mod 49 = 7a+b, t = 7si+sj; valid iff the shifted
    # window position stays on the 7x7 grid in both axes.
    m = np.zeros((ROWS, HW), dtype=np.float32)
    for p in range(ROWS):
        a, b = divmod(p % HW, 7)
        for t in range(HW):
            si, sj = divmod(t, 7)
            if 0 <= a + si - 3 < 7 and 0 <= b + sj - 3 < 7:
                m[p, t] = 1.0
    return m


def build():
    import ml_dtypes

    nc = bacc.Bacc(
        "TRN2", target_bir_lowering=False, debug=False, num_devices=NCORES
    )
    x_d = nc.dram_tensor("x", (B, C, H, W), F32, kind="ExternalInput")
    wk_d = nc.dram_tensor("Wk", (P,), F32, kind="ExternalInput")
    bk_d = nc.dram_tensor("bk", (P,), F32, kind="ExternalInput")
    ba_d = nc.dram_tensor("b_adap", (P,), F32, kind="ExternalInput")
    out_d = nc.dram_tensor("out", (B, P, H, W), F32, kind="ExternalOutput")
    # staging: inline zero constants -> pads are zero at model load and the
    # per-run dump only ever writes the data region (cols 24:122)
    stag = [
        nc.inline_tensor(
            np.zeros(SROW * ROWS, dtype=np.float32), name=f"stag{nh}"
        )
        for nh in range(2)
    ]

    with tile.TileContext(nc) as tc:
        with (
            tc.tile_pool(name="sb", bufs=1) as sb,
            tc.tile_pool(name="ps", bufs=1, space="PSUM") as ps,
        ):
            xsb = sb.tile([128, 2, B, HW], F32)       # x, channels on partitions
            xbf = sb.tile([128, 2, B, ROWS], BF16)    # [x | ones] per (ck, n)
            gsb = sb.tile([ROWS, 2, HF], F32)         # PSUM rows staged per half
            T = sb.tile([ROWS, 2, HF], F32)           # gathered bands per half
            Tbf = sb.tile([ROWS, 2, HF], BF16)        # masked + cast
            mk = sb.tile([ROWS, HW], F32)             # band validity mask
            prm = sb.tile([2, P], F32)                # Wk; bk
            prmbf = sb.tile([2, P], BF16)
            selT = sb.tile([2, ROWS], BF16)           # row-region selector
            Qbf = sb.tile([ROWS, P], BF16)
            badap = sb.tile([128, 2], F32)            # b_adap, chunked
            ysb = sb.tile([128, 2, 2, HF], F32)       # (p, ck, nh, (n2, t))

            GX_ps = ps.tile([ROWS, FREE], F32)
            Q_ps = ps.tile([ROWS, P], F32)
            Y_ps = [ps.tile([128, FREE], F32, name=f"y{k}") for k in range(2)]

            sel_np = np.zeros((2, ROWS), dtype=ml_dtypes.bfloat16)
            sel_np[0, 0:HW] = 1.0
            sel_np[1, HW:ROWS] = 1.0
            sel_d = nc.inline_tensor(sel_np, name="sel_const")
            mask_d = nc.inline_tensor(_mask_np(), name="mask_const")

            # prologue constants / params on the software DGE (off critical path)
            nc.gpsimd.dma_start(mk[:], mask_d[:])
            nc.gpsimd.dma_start(selT[:], sel_d[:])
            nc.gpsimd.dma_start(prm[0:1, :], wk_d.ap().unsqueeze(0))
            nc.gpsimd.dma_start(prm[1:2, :], bk_d.ap().unsqueeze(0))
            nc.gpsimd.dma_start(badap[:], AP(ba_d, 0, [[1, 128], [128, 2]]))

            # ones region of the stationary; x casts fill cols 0:49
            nc.vector.memset(xbf[:, :, :, HW:ROWS], 1.0)

            # x -> SBUF, channels on partitions; first half first on both queues
            xr = x_d.ap().rearrange("n (k c) h w -> k c n (h w)", k=2)
            nc.sync.dma_start(xsb[:, 0, 0:2], xr[0, :, 0:2])
            nc.scalar.dma_start(xsb[:, 1, 0:2], xr[1, :, 0:2])
            nc.sync.dma_start(xsb[:, 0, 2:4], xr[0, :, 2:4])
            nc.scalar.dma_start(xsb[:, 1, 2:4], xr[1, :, 2:4])

            # Q = selT^T @ [Wk; bk]: rows 0:49 Wk, rows 49:98 bk
            nc.vector.tensor_copy(prmbf[:], prm[:])
            nc.tensor.matmul(Q_ps[:], selT[:], prmbf[:], start=True, stop=True)
            nc.vector.tensor_copy(Qbf[:], Q_ps[:])

            for ck in range(2):
                for nh in range(2):
                    ns = slice(2 * nh, 2 * nh + 2)
                    nc.vector.tensor_copy(xbf[:, ck, ns, 0:HW], xsb[:, ck, ns])

            outr = out_d.ap().rearrange("n (k p) h w -> k p n (h w)", k=2)
            mkb = AP(mk.tensor, 0, [[HW, ROWS], [0, 2], [1, HW]])
            for nh in range(2):
                ns = slice(2 * nh, 2 * nh + 2)
                for n in range(2 * nh, 2 * nh + 2):
                    for ck in range(2):
                        nc.tensor.matmul(
                            GX_ps[:, n * HW : (n + 1) * HW],
                            xbf[:, ck, n],
                            xbf[:, ck, n, 0:HW],
                            start=(ck == 0),
                            stop=(ck == 1),
                        )
                nc.vector.tensor_copy(
                    gsb[:, nh], GX_ps[:, 2 * nh * HW : (2 * nh + 2) * HW]
                )
                (nc.sync if nh == 0 else nc.scalar).dma_start(
                    AP(stag[nh], 24, [[SROW, ROWS], [1, HF]]), gsb[:, nh]
                )
                # one gather per region: T[reg*49+r, nh, i] = stag[nh][193*(reg*49+r) - reg*49*... ]
                # flat: stag[192*p + r + i] with p = reg*49 + r
                for reg in range(2):
                    src = AP(stag[nh], SROW * HW * reg, [[SROW + 1, HW], [1, HF]])
                    dst = AP(
                        T.tensor,
                        (HW * reg) * (2 * HF) + nh * HF,
                        [[2 * HF, HW], [1, HF]],
                    )
                    (nc.sync if reg == 0 else nc.scalar).dma_start(dst, src)
                nc.vector.tensor_tensor(
                    Tbf[:, nh].rearrange("p (n t) -> p n t", n=2),
                    T[:, nh].rearrange("p (n t) -> p n t", n=2),
                    mkb,
                    op=mybir.AluOpType.mult,
                )
                for pk in range(2):
                    nc.tensor.matmul(
                        Y_ps[pk][:, nh * HF : (nh + 1) * HF],
                        Qbf[:, pk * 128 : (pk + 1) * 128],
                        Tbf[:, nh],
                        start=True,
                        stop=True,
                    )
                    nc.vector.tensor_scalar_add(
                        ysb[:, pk, nh],
                        Y_ps[pk][:, nh * HF : (nh + 1) * HF],
                        badap[:, pk : pk + 1],
                    )

            # one output DMA per chunk (fewer completion waits at the end)
            for pk in range(2):
                (nc.sync if pk == 0 else nc.scalar).dma_start(
                    outr[pk][:],
                    ysb[:, pk].rearrange("p h (n t) -> p (h n) t", n=2),
                )

    nc.compile()
    return nc


def kernel(x, Wk, bk, b_adap):
    global last_exec_time_ns
    if "nc" not in _cached:
        _cached["nc"] = build()
    nc = _cached["nc"]

    x = np.ascontiguousarray(x, dtype=np.float32)
    Wk = np.ascontiguousarray(Wk, dtype=np.float32)
    bk = np.ascontiguousarray(bk, dtype=np.float32)
    b_adap = np.ascontiguousarray(b_adap, dtype=np.float32)

    in_maps = [
        {"x": x[i * B : (i + 1) * B], "Wk": Wk, "bk": bk, "b_adap": b_adap}
        for i in range(NCORES)
    ]
    res = run_bass_kernel_spmd(
        nc,
        in_maps,
        core_ids=list(range(NCORES)),
        trace=bool(os.environ.get("KERNEL_TRACE")),
    )
    last_exec_time_ns = res.exec_time_ns
    out = np.concatenate(
        [res.results[i]["out"].reshape(B, P, H, W) for i in range(NCORES)], axis=0
    )
    return out


# revision 22
# speedup vs baseline: 1.0536x; 1.0536x over previous
"""AdaptiveKernelFC Trainium2 kernel (8-core data parallel), v3.

Math: the reference builds per-sample filters w[n,p,c,kh,kw] =
x[n,c,kh,kw]*Wk[p] + bk[p] and convolves x[n] with them (7x7 kernel ==
feature map size, pad 3).  The conv factors exactly:

    y[n,p,i,j] = Wk[p]*S1[n,i,j] + bk[p]*S2[n,i,j] + b_adap[p]

with S1 the 2D autocorrelation of x[n] (summed over channels) and S2
the 7x7 box-filter correlation of the channel sums.  Both come from one
fused matmul pair per sample with stationary [x | ones]: PSUM rows 0:49
hold the spatial Gram matrix G[r, q], rows 49:98 hold the channel sums
xs[q] replicated.

The diagonal band T[p, (n, t)] = row p's value at column (r-24)+t
(r = p mod 49) is produced by a DRAM staging round trip (SBUF DMAs
cannot mix partition and element steps; DRAM is flat): dump rows at
stride 192 starting at column 24, then ONE gather per (region, half)
with row stride 193 -- the +1 slope realizes the per-row shift.  The
staging tensors are inline zero constants, so the out-of-band positions
the gather sweeps through are always finite zeros/neighbor data; a
single mask-multiply (f32 mask broadcast over samples, bf16 output)
zeroes the wrapped positions and casts for the TensorEngine in one op.

The final stage contracts T directly against Q (98, 256) where
Q[p, :] = Wk if p < 49 else bk (built once off the critical path by a
tiny selector matmul), so the selector reduction, bias staging and R
assembly of earlier versions all disappear:

    y[ck*128+p, (n,s)] = (Q[:, ck]^T Tbf)[p, (n,s)] + b_adap
    (b_adap added by the PSUM->SBUF tensor_scalar_add move).

Everything is split into two sample-halves so the second half's Gram
matmuls and round trip overlap the first half's output pipeline.

Sharding: pure data parallel, batch N=32 split 4 samples/core across 8
cores; params replicated; outputs concatenated.
"""

import os
import numpy as np

import concourse.bass as bass
import concourse.bacc as bacc
import concourse.mybir as mybir
import concourse.tile as tile
from concourse.ap import AP
from concourse.bass_utils import run_bass_kernel_spmd

N, C, H, W = 32, 256, 7, 7
P = 256
NCORES = 8
B = N // NCORES          # samples per core
HW = H * W               # 49
ROWS = 2 * HW            # 98 PSUM rows (G region then xs region)
FREE = B * HW            # 196 free columns (n, t)
HF = 2 * HW              # 98 free columns per half
SROW = 192               # staging row stride (f32 elems); band reads stay in-row
F32 = mybir.dt.float32
BF16 = mybir.dt.bfloat16

_cached = {}
last_exec_time_ns = None


def _mask_np():
    # mask[p, t]: r = p mod 49 = 7a+b, t = 7si+sj; valid iff the shifted
    # window position stays on the 7x7 grid in both axes.
    m = np.zeros((ROWS, HW), dtype=np.float32)
    for p in range(ROWS):
        a, b = divmod(p % HW, 7)
        for t in range(HW):
            si, sj = divmod(t, 7)
            if 0 <= a + si - 3 < 7 and 0 <= b + sj - 3 < 7:
                m[p, t] = 1.0
    return m


def build():
    import ml_dtypes

    nc = bacc.Bacc(
        "TRN2", target_bir_lowering=False, debug=False, num_devices=1
    )
    x_d = nc.dram_tensor("x", (B, C, H, W), F32, kind="ExternalInput")
    wk_d = nc.dram_tensor("Wk", (P,), F32, kind="ExternalInput")
    bk_d = nc.dram_tensor("bk", (P,), F32, kind="ExternalInput")
    ba_d = nc.dram_tensor("b_adap", (P,), F32, kind="ExternalInput")
    out_d = nc.dram_tensor("out", (B, P, H, W), F32, kind="ExternalOutput")
    # staging: inline zero constants -> pads are zero at model load and the
    # per-run dump only ever writes the data region (cols 24:122)
    stag = [
        nc.inline_tensor(
            np.zeros(SROW * ROWS, dtype=np.float32), name=f"stag{nh}"
        )
        for nh in range(2)
    ]

    with tile.TileContext(nc) as tc:
        with (
            tc.tile_pool(name="sb", bufs=1) as sb,
            tc.tile_pool(name="ps", bufs=1, space="PSUM") as ps,
        ):
            xsb = sb.tile([128, 2, B, HW], F32)       # x, channels on partitions
            xbf = sb.tile([128, 2, B, ROWS], BF16)    # [x | ones] per (ck, n)
            gsb = sb.tile([ROWS, 2, HF], F32)         # PSUM rows staged per half
            T = sb.tile([ROWS, 2, HF], F32)           # gathered bands per half
            Tbf = sb.tile([ROWS, 2, HF], BF16)        # masked + cast
            mk = sb.tile([ROWS, HW], F32)             # band validity mask
            prm = sb.tile([2, P], F32)                # Wk; bk
            prmbf = sb.tile([2, P], BF16)
            selT = sb.tile([2, ROWS], BF16)           # row-region selector
            Qbf = sb.tile([ROWS, P], BF16)
            badap = sb.tile([128, 2], F32)            # b_adap, chunked
            ysb = sb.tile([128, 2, 2, HF], F32)       # (p, ck, nh, (n2, t))

            GX_ps = ps.tile([ROWS, FREE], F32)
            Q_ps = ps.tile([ROWS, P], F32)
            Y_ps = [ps.tile([128, FREE], F32, name=f"y{k}") for k in range(2)]

            sel_np = np.zeros((2, ROWS), dtype=ml_dtypes.bfloat16)
            sel_np[0, 0:HW] = 1.0
            sel_np[1, HW:ROWS] = 1.0
            sel_d = nc.inline_tensor(sel_np, name="sel_const")
            mask_d = nc.inline_tensor(_mask_np(), name="mask_const")

            # prologue constants / params on the software DGE (off critical path)
            nc.gpsimd.dma_start(mk[:], mask_d[:])
            nc.gpsimd.dma_start(selT[:], sel_d[:])
            nc.gpsimd.dma_start(prm[0:1, :], wk_d.ap().unsqueeze(0))
            nc.gpsimd.dma_start(prm[1:2, :], bk_d.ap().unsqueeze(0))
            nc.gpsimd.dma_start(badap[:], AP(ba_d, 0, [[1, 128], [128, 2]]))

            # ones region of the stationary; x casts fill cols 0:49
            nc.vector.memset(xbf[:, :, :, HW:ROWS], 1.0)

            # x -> SBUF, channels on partitions; first half first on both queues
            xr = x_d.ap().rearrange("n (k c) h w -> k c n (h w)", k=2)
            nc.sync.dma_start(xsb[:, 0, 0:2], xr[0, :, 0:2])
            nc.scalar.dma_start(xsb[:, 1, 0:2], xr[1, :, 0:2])
            nc.sync.dma_start(xsb[:, 0, 2:4], xr[0, :, 2:4])
            nc.scalar.dma_start(xsb[:, 1, 2:4], xr[1, :, 2:4])

            # Q = selT^T @ [Wk; bk]: rows 0:49 Wk, rows 49:98 bk
            nc.vector.tensor_copy(prmbf[:], prm[:])
            nc.tensor.matmul(Q_ps[:], selT[:], prmbf[:], start=True, stop=True)
            nc.vector.tensor_copy(Qbf[:], Q_ps[:])

            for ck in range(2):
                for nh in range(2):
                    ns = slice(2 * nh, 2 * nh + 2)
                    nc.vector.tensor_copy(xbf[:, ck, ns, 0:HW], xsb[:, ck, ns])

            outr = out_d.ap().rearrange("n (k p) h w -> k p n (h w)", k=2)
            mkb = AP(mk.tensor, 0, [[HW, ROWS], [0, 2], [1, HW]])
            for nh in range(2):
                ns = slice(2 * nh, 2 * nh + 2)
                for n in range(2 * nh, 2 * nh + 2):
                    for ck in range(2):
                        nc.tensor.matmul(
                            GX_ps[:, n * HW : (n + 1) * HW],
                            xbf[:, ck, n],
                            xbf[:, ck, n, 0:HW],
                            start=(ck == 0),
                            stop=(ck == 1),
                        )
                nc.vector.tensor_copy(
                    gsb[:, nh], GX_ps[:, 2 * nh * HW : (2 * nh + 2) * HW]
                )
                (nc.sync if nh == 0 else nc.scalar).dma_start(
                    AP(stag[nh], 24, [[SROW, ROWS], [1, HF]]), gsb[:, nh]
                )
                # one gather per region: T[reg*49+r, nh, i] = stag[nh][193*(reg*49+r) - reg*49*... ]
                # flat: stag[192*p + r + i] with p = reg*49 + r
                for reg in range(2):
                    src = AP(stag[nh], SROW * HW * reg, [[SROW + 1, HW], [1, HF]])
                    dst = AP(
                        T.tensor,
                        (HW * reg) * (2 * HF) + nh * HF,
                        [[2 * HF, HW], [1, HF]],
                    )
                    (nc.sync if reg == 0 else nc.scalar).dma_start(dst, src)
                nc.vector.tensor_tensor(
                    Tbf[:, nh].rearrange("p (n t) -> p n t", n=2),
                    T[:, nh].rearrange("p (n t) -> p n t", n=2),
                    mkb,
                    op=mybir.AluOpType.mult,
                )
                for pk in range(2):
                    nc.tensor.matmul(
                        Y_ps[pk][:, nh * HF : (nh + 1) * HF],
                        Qbf[:, pk * 128 : (pk + 1) * 128],
                        Tbf[:, nh],
                        start=True,
                        stop=True,
                    )
                    nc.vector.tensor_scalar_add(
                        ysb[:, pk, nh],
                        Y_ps[pk][:, nh * HF : (nh + 1) * HF],
                        badap[:, pk : pk + 1],
                    )
                    (nc.sync if pk == 0 else nc.scalar).dma_start(
                        outr[pk][:, ns],
                        ysb[:, pk, nh].rearrange("p (n t) -> p n t", n=2),
                    )

    nc.compile()
    return nc


def kernel(x, Wk, bk, b_adap):
    global last_exec_time_ns
    if "nc" not in _cached:
        _cached["nc"] = build()
    nc = _cached["nc"]

    x = np.ascontiguousarray(x, dtype=np.float32)
    Wk = np.ascontiguousarray(Wk, dtype=np.float32)
    bk = np.ascontiguousarray(bk, dtype=np.float32)
    b_adap = np.ascontiguousarray(b_adap, dtype=np.float32)

    in_maps = [
        {"x": x[i * B : (i + 1) * B], "Wk": Wk, "bk": bk, "b_adap": b_adap}
        for i in range(NCORES)
    ]
    res = run_bass_kernel_spmd(
        nc,
        in_maps,
        core_ids=list(range(NCORES)),
        trace=bool(os.environ.get("KERNEL_TRACE")),
    )
    last_exec_time_ns = res.exec_time_ns
    out = np.concatenate(
        [res.results[i]["out"].reshape(B, P, H, W) for i in range(NCORES)], axis=0
    )
    return out


# revision 26
# speedup vs baseline: 1.1090x; 1.0526x over previous
"""AdaptiveKernelFC Trainium2 kernel (8-core data parallel), v3.

Math: the reference builds per-sample filters w[n,p,c,kh,kw] =
x[n,c,kh,kw]*Wk[p] + bk[p] and convolves x[n] with them (7x7 kernel ==
feature map size, pad 3).  The conv factors exactly:

    y[n,p,i,j] = Wk[p]*S1[n,i,j] + bk[p]*S2[n,i,j] + b_adap[p]

with S1 the 2D autocorrelation of x[n] (summed over channels) and S2
the 7x7 box-filter correlation of the channel sums.  Both come from one
fused matmul pair per sample with stationary [x | ones]: PSUM rows 0:49
hold the spatial Gram matrix G[r, q], rows 49:98 hold the channel sums
xs[q] replicated.

The diagonal band T[p, (n, t)] = row p's value at column (r-24)+t
(r = p mod 49) is produced by a DRAM staging round trip (SBUF DMAs
cannot mix partition and element steps; DRAM is flat): dump rows at
stride 192 starting at column 24, then ONE gather per (region, half)
with row stride 193 -- the +1 slope realizes the per-row shift.  The
staging tensors are inline zero constants, so the out-of-band positions
the gather sweeps through are always finite zeros/neighbor data; a
single mask-multiply (f32 mask broadcast over samples, bf16 output)
zeroes the wrapped positions and casts for the TensorEngine in one op.

The final stage contracts T directly against Q (98, 256) where
Q[p, :] = Wk if p < 49 else bk (built once off the critical path by a
tiny selector matmul), so the selector reduction, bias staging and R
assembly of earlier versions all disappear:

    y[ck*128+p, (n,s)] = (Q[:, ck]^T Tbf)[p, (n,s)] + b_adap
    (b_adap added by the PSUM->SBUF tensor_scalar_add move).

Everything is split into two sample-halves so the second half's Gram
matmuls and round trip overlap the first half's output pipeline.

Sharding: pure data parallel, batch N=32 split 4 samples/core across 8
cores; params replicated; outputs concatenated.
"""

import os
import numpy as np

import concourse.bass as bass
import concourse.bacc as bacc
import concourse.mybir as mybir
import concourse.tile as tile
from concourse.ap import AP
from concourse.bass_utils import run_bass_kernel_spmd

N, C, H, W = 32, 256, 7, 7
P = 256
NCORES = 8
B = N // NCORES          # samples per core
HW = H * W               # 49
ROWS = 2 * HW            # 98 PSUM rows (G region then xs region)
FREE = B * HW            # 196 free columns (n, t)
HF = 2 * HW              # 98 free columns per half
SROW = 192               # staging row stride (f32 elems); band reads stay in-row
F32 = mybir.dt.float32
BF16 = mybir.dt.bfloat16

_cached = {}
last_exec_time_ns = None


def _mask_np():
    # mask[p, t]: r = p mod 49 = 7a+b, t = 7si+sj; valid iff the shifted
    # window position stays on the 7x7 grid in both axes.
    m = np.zeros((ROWS, HW), dtype=np.float32)
    for p in range(ROWS):
        a, b = divmod(p % HW, 7)
        for t in range(HW):
            si, sj = divmod(t, 7)
            if 0 <= a + si - 3 < 7 and 0 <= b + sj - 3 < 7:
                m[p, t] = 1.0
    return m


def build():
    import ml_dtypes

    nc = bacc.Bacc(
        "TRN2", target_bir_lowering=False, debug=False, num_devices=NCORES
    )
    x_d = nc.dram_tensor("x", (B, C, H, W), F32, kind="ExternalInput")
    wk_d = nc.dram_tensor("Wk", (P,), F32, kind="ExternalInput")
    bk_d = nc.dram_tensor("bk", (P,), F32, kind="ExternalInput")
    ba_d = nc.dram_tensor("b_adap", (P,), F32, kind="ExternalInput")
    out_d = nc.dram_tensor("out", (B, P, H, W), F32, kind="ExternalOutput")
    # staging: inline zero constants -> pads are zero at model load and the
    # per-run dump only ever writes the data region (cols 24:122)
    stag = [
        nc.inline_tensor(
            np.zeros(SROW * ROWS, dtype=ml_dtypes.bfloat16), name=f"stag{nh}"
        )
        for nh in range(2)
    ]

    with tile.TileContext(nc) as tc:
        with (
            tc.tile_pool(name="sb", bufs=1) as sb,
            tc.tile_pool(name="ps", bufs=1, space="PSUM") as ps,
        ):
            xsb = sb.tile([128, 2, B, HW], F32)       # x, channels on partitions
            xbf = sb.tile([128, 2, B, ROWS], BF16)    # [x | ones] per (ck, n)
            gsb = sb.tile([ROWS, 2, HF], BF16)        # PSUM rows staged per half
            T = sb.tile([ROWS, 2, HF], BF16)          # gathered bands per half
            Tbf = sb.tile([ROWS, 2, HF], BF16)        # masked + cast
            mk = sb.tile([ROWS, HW], F32)             # band validity mask
            prm = sb.tile([2, P], F32)                # Wk; bk
            prmbf = sb.tile([2, P], BF16)
            selT = sb.tile([2, ROWS], BF16)           # row-region selector
            Qbf = sb.tile([ROWS, P], BF16)
            badap = sb.tile([128, 2], F32)            # b_adap, chunked
            ysb = sb.tile([128, 2, 2, HF], F32)       # (p, ck, nh, (n2, t))

            GX_ps = ps.tile([ROWS, FREE], F32)
            Q_ps = ps.tile([ROWS, P], F32)
            Y_ps = [ps.tile([128, FREE], F32, name=f"y{k}") for k in range(2)]

            sel_np = np.zeros((2, ROWS), dtype=ml_dtypes.bfloat16)
            sel_np[0, 0:HW] = 1.0
            sel_np[1, HW:ROWS] = 1.0
            sel_d = nc.inline_tensor(sel_np, name="sel_const")
            mask_d = nc.inline_tensor(_mask_np(), name="mask_const")

            # prologue constants / params on the software DGE (off critical
            # path); Q-path inputs first so the Q matmul isn't gated late
            nc.gpsimd.dma_start(prm[0:1, :], wk_d.ap().unsqueeze(0))
            nc.gpsimd.dma_start(prm[1:2, :], bk_d.ap().unsqueeze(0))
            nc.gpsimd.dma_start(selT[:], sel_d[:])
            nc.gpsimd.dma_start(badap[:], AP(ba_d, 0, [[1, 128], [128, 2]]))
            nc.gpsimd.dma_start(mk[:], mask_d[:])

            # ones region of the stationary; x casts fill cols 0:49
            nc.vector.memset(xbf[:, :, :, HW:ROWS], 1.0)

            # x -> SBUF, channels on partitions; first half first on both queues
            xr = x_d.ap().rearrange("n (k c) h w -> k c n (h w)", k=2)
            nc.sync.dma_start(xsb[:, 0, 0:2], xr[0, :, 0:2])
            nc.scalar.dma_start(xsb[:, 1, 0:2], xr[1, :, 0:2])
            nc.sync.dma_start(xsb[:, 0, 2:4], xr[0, :, 2:4])
            nc.scalar.dma_start(xsb[:, 1, 2:4], xr[1, :, 2:4])

            # Q = selT^T @ [Wk; bk]: rows 0:49 Wk, rows 49:98 bk
            nc.vector.tensor_copy(prmbf[:], prm[:])
            nc.tensor.matmul(Q_ps[:], selT[:], prmbf[:], start=True, stop=True)
            nc.vector.tensor_copy(Qbf[:], Q_ps[:])

            for ck in range(2):
                for nh in range(2):
                    ns = slice(2 * nh, 2 * nh + 2)
                    nc.vector.tensor_copy(xbf[:, ck, ns, 0:HW], xsb[:, ck, ns])

            outr = out_d.ap().rearrange("n (k p) h w -> k p n (h w)", k=2)
            mkb = AP(mk.tensor, 0, [[HW, ROWS], [0, 2], [1, HW]])
            for nh in range(2):
                ns = slice(2 * nh, 2 * nh + 2)
                for n in range(2 * nh, 2 * nh + 2):
                    for ck in range(2):
                        nc.tensor.matmul(
                            GX_ps[:, n * HW : (n + 1) * HW],
                            xbf[:, ck, n],
                            xbf[:, ck, n, 0:HW],
                            start=(ck == 0),
                            stop=(ck == 1),
                        )
                nc.vector.tensor_copy(
                    gsb[:, nh], GX_ps[:, 2 * nh * HW : (2 * nh + 2) * HW]
                )
                (nc.sync if nh == 0 else nc.scalar).dma_start(
                    AP(stag[nh], 24, [[SROW, ROWS], [1, HF]]), gsb[:, nh]
                )
                # one gather per region: T[reg*49+r, nh, i] = stag[nh][193*(reg*49+r) - reg*49*... ]
                # flat: stag[192*p + r + i] with p = reg*49 + r
                for reg in range(2):
                    src = AP(stag[nh], SROW * HW * reg, [[SROW + 1, HW], [1, HF]])
                    dst = AP(
                        T.tensor,
                        (HW * reg) * (2 * HF) + nh * HF,
                        [[2 * HF, HW], [1, HF]],
                    )
                    (nc.sync if reg == 0 else nc.scalar).dma_start(dst, src)
                nc.vector.tensor_tensor(
                    Tbf[:, nh].rearrange("p (n t) -> p n t", n=2),
                    T[:, nh].rearrange("p (n t) -> p n t", n=2),
                    mkb,
                    op=mybir.AluOpType.mult,
                )
                for pk in range(2):
                    nc.tensor.matmul(
                        Y_ps[pk][:, nh * HF : (nh + 1) * HF],
                        Qbf[:, pk * 128 : (pk + 1) * 128],
                        Tbf[:, nh],
                        start=True,
                        stop=True,
                    )
                    nc.vector.tensor_scalar_add(
                        ysb[:, pk, nh],
                        Y_ps[pk][:, nh * HF : (nh + 1) * HF],
                        badap[:, pk : pk + 1],
                    )
                    (nc.sync if pk == 0 else nc.scalar).dma_start(
                        outr[pk][:, ns],
                        ysb[:, pk, nh].rearrange("p (n t) -> p n t", n=2),
                    )

    nc.compile()
    return nc


def kernel(x, Wk, bk, b_adap):
    global last_exec_time_ns
    if "nc" not in _cached:
        _cached["nc"] = build()
    nc = _cached["nc"]

    x = np.ascontiguousarray(x, dtype=np.float32)
    Wk = np.ascontiguousarray(Wk, dtype=np.float32)
    bk = np.ascontiguousarray(bk, dtype=np.float32)
    b_adap = np.ascontiguousarray(b_adap, dtype=np.float32)

    in_maps = [
        {"x": x[i * B : (i + 1) * B], "Wk": Wk, "bk": bk, "b_adap": b_adap}
        for i in range(NCORES)
    ]
    res = run_bass_kernel_spmd(
        nc,
        in_maps,
        core_ids=list(range(NCORES)),
        trace=bool(os.environ.get("KERNEL_TRACE")),
    )
    last_exec_time_ns = res.exec_time_ns
    out = np.concatenate(
        [res.results[i]["out"].reshape(B, P, H, W) for i in range(NCORES)], axis=0
    )
    return out


# revision 30
# speedup vs baseline: 1.1110x; 1.0017x over previous
"""AdaptiveKernelFC Trainium2 kernel (8-core data parallel), v3.

Math: the reference builds per-sample filters w[n,p,c,kh,kw] =
x[n,c,kh,kw]*Wk[p] + bk[p] and convolves x[n] with them (7x7 kernel ==
feature map size, pad 3).  The conv factors exactly:

    y[n,p,i,j] = Wk[p]*S1[n,i,j] + bk[p]*S2[n,i,j] + b_adap[p]

with S1 the 2D autocorrelation of x[n] (summed over channels) and S2
the 7x7 box-filter correlation of the channel sums.  Both come from one
fused matmul pair per sample with stationary [x | ones]: PSUM rows 0:49
hold the spatial Gram matrix G[r, q], rows 49:98 hold the channel sums
xs[q] replicated.

The diagonal band T[p, (n, t)] = row p's value at column (r-24)+t
(r = p mod 49) is produced by a DRAM staging round trip (SBUF DMAs
cannot mix partition and element steps; DRAM is flat): dump rows at
stride 192 starting at column 24, then ONE gather per (region, half)
with row stride 193 -- the +1 slope realizes the per-row shift.  The
staging tensors are inline zero constants, so the out-of-band positions
the gather sweeps through are always finite zeros/neighbor data; a
single mask-multiply (f32 mask broadcast over samples, bf16 output)
zeroes the wrapped positions and casts for the TensorEngine in one op.

The final stage contracts T directly against Q (98, 256) where
Q[p, :] = Wk if p < 49 else bk (built once off the critical path by a
tiny selector matmul), so the selector reduction, bias staging and R
assembly of earlier versions all disappear:

    y[ck*128+p, (n,s)] = (Q[:, ck]^T Tbf)[p, (n,s)] + b_adap
    (b_adap added by the PSUM->SBUF tensor_scalar_add move).

Everything is split into two sample-halves so the second half's Gram
matmuls and round trip overlap the first half's output pipeline.

Sharding: pure data parallel, batch N=32 split 4 samples/core across 8
cores; params replicated; outputs concatenated.
"""

import os
import numpy as np

import concourse.bass as bass
import concourse.bacc as bacc
import concourse.mybir as mybir
import concourse.tile as tile
from concourse.ap import AP
from concourse.bass_utils import run_bass_kernel_spmd

N, C, H, W = 32, 256, 7, 7
P = 256
NCORES = 8
B = N // NCORES          # samples per core
HW = H * W               # 49
ROWS = 2 * HW            # 98 PSUM rows (G region then xs region)
FREE = B * HW            # 196 free columns (n, t)
HF = 2 * HW              # 98 free columns per half
SROW = 192               # staging row stride (f32 elems); band reads stay in-row
F32 = mybir.dt.float32
BF16 = mybir.dt.bfloat16

_cached = {}
last_exec_time_ns = None


def _mask_np():
    # mask[p, t]: r = p mod 49 = 7a+b, t = 7si+sj; valid iff the shifted
    # window position stays on the 7x7 grid in both axes.
    m = np.zeros((ROWS, HW), dtype=np.float32)
    for p in range(ROWS):
        a, b = divmod(p % HW, 7)
        for t in range(HW):
            si, sj = divmod(t, 7)
            if 0 <= a + si - 3 < 7 and 0 <= b + sj - 3 < 7:
                m[p, t] = 1.0
    return m


def build():
    import ml_dtypes

    nc = bacc.Bacc(
        "TRN2", target_bir_lowering=False, debug=False, num_devices=NCORES
    )
    x_d = nc.dram_tensor("x", (B, C, H, W), F32, kind="ExternalInput")
    wk_d = nc.dram_tensor("Wk", (P,), F32, kind="ExternalInput")
    bk_d = nc.dram_tensor("bk", (P,), F32, kind="ExternalInput")
    ba_d = nc.dram_tensor("b_adap", (P,), F32, kind="ExternalInput")
    out_d = nc.dram_tensor("out", (B, P, H, W), F32, kind="ExternalOutput")
    # staging: inline zero constants -> pads are zero at model load and the
    # per-run dump only ever writes the data region (cols 24:122)
    stag = [
        nc.inline_tensor(
            np.zeros(SROW * ROWS, dtype=ml_dtypes.bfloat16), name=f"stag{nh}"
        )
        for nh in range(2)
    ]

    with tile.TileContext(nc) as tc:
        with (
            tc.tile_pool(name="sb", bufs=1) as sb,
            tc.tile_pool(name="ps", bufs=1, space="PSUM") as ps,
        ):
            xsb = sb.tile([128, 2, B, HW], F32)       # x, channels on partitions
            xbf = sb.tile([128, 2, B, ROWS], BF16)    # [x | ones] per (ck, n)
            gsb = sb.tile([ROWS, 2, HF], BF16)        # PSUM rows staged per half
            T = sb.tile([ROWS, 2, HF], BF16)          # gathered bands per half
            Tbf = sb.tile([ROWS, 2, HF], BF16)        # masked + cast
            mk = sb.tile([ROWS, HW], F32)             # band validity mask
            prm = sb.tile([2, P], F32)                # Wk; bk
            prmbf = sb.tile([2, P], BF16)
            selT = sb.tile([2, ROWS], BF16)           # row-region selector
            Qbf = sb.tile([ROWS, P], BF16)
            badap = sb.tile([128, 2], F32)            # b_adap, chunked
            ysb = sb.tile([128, 2, 2, HF], F32)       # (p, ck, nh, (n2, t))

            GX_ps = ps.tile([ROWS, FREE], F32)
            Q_ps = ps.tile([ROWS, P], F32)
            Y_ps = [ps.tile([128, FREE], F32, name=f"y{k}") for k in range(2)]

            sel_np = np.zeros((2, ROWS), dtype=ml_dtypes.bfloat16)
            sel_np[0, 0:HW] = 1.0
            sel_np[1, HW:ROWS] = 1.0
            sel_d = nc.inline_tensor(sel_np, name="sel_const")
            mask_d = nc.inline_tensor(_mask_np(), name="mask_const")

            # prologue constants / params on the software DGE (off critical
            # path); Q-path inputs first so the Q matmul isn't gated late
            nc.gpsimd.dma_start(prm[0:1, :], wk_d.ap().unsqueeze(0))
            nc.gpsimd.dma_start(prm[1:2, :], bk_d.ap().unsqueeze(0))
            nc.gpsimd.dma_start(selT[:], sel_d[:])
            nc.gpsimd.dma_start(badap[:], AP(ba_d, 0, [[1, 128], [128, 2]]))
            nc.gpsimd.dma_start(mk[:], mask_d[:])

            # ones region of the stationary; x casts fill cols 0:49
            nc.vector.memset(xbf[:, :, :, HW:ROWS], 1.0)

            # x -> SBUF, channels on partitions; one DMA per queue so the
            # completion semaphore fires at queue position 1 for everything
            xr = x_d.ap().rearrange("n (k c) h w -> k c n (h w)", k=2)
            nc.sync.dma_start(xsb[:, 0], xr[0])
            nc.scalar.dma_start(xsb[:, 1], xr[1])

            # Q = selT^T @ [Wk; bk]: rows 0:49 Wk, rows 49:98 bk
            # (copies on GpSimd to keep the Vector engine free)
            nc.gpsimd.tensor_copy(prmbf[:], prm[:])
            nc.tensor.matmul(Q_ps[:], selT[:], prmbf[:], start=True, stop=True)

            outr = out_d.ap().rearrange("n (k p) h w -> k p n (h w)", k=2)
            nc.vector.tensor_copy(Qbf[:], Q_ps[:])
            mkb = AP(mk.tensor, 0, [[HW, ROWS], [0, 2], [1, HW]])
            for nh in range(2):
                ns = slice(2 * nh, 2 * nh + 2)
                for ck in range(2):
                    nc.vector.tensor_copy(xbf[:, ck, ns, 0:HW], xsb[:, ck, ns])
                for n in range(2 * nh, 2 * nh + 2):
                    for ck in range(2):
                        nc.tensor.matmul(
                            GX_ps[:, n * HW : (n + 1) * HW],
                            xbf[:, ck, n],
                            xbf[:, ck, n, 0:HW],
                            start=(ck == 0),
                            stop=(ck == 1),
                        )
                nc.vector.tensor_copy(
                    gsb[:, nh], GX_ps[:, 2 * nh * HW : (2 * nh + 2) * HW]
                )
                (nc.sync if nh == 0 else nc.scalar).dma_start(
                    AP(stag[nh], 24, [[SROW, ROWS], [1, HF]]), gsb[:, nh]
                )
                # one gather per region: T[reg*49+r, nh, i] = stag[nh][193*(reg*49+r) - reg*49*... ]
                # flat: stag[192*p + r + i] with p = reg*49 + r
                for reg in range(2):
                    src = AP(stag[nh], SROW * HW * reg, [[SROW + 1, HW], [1, HF]])
                    dst = AP(
                        T.tensor,
                        (HW * reg) * (2 * HF) + nh * HF,
                        [[2 * HF, HW], [1, HF]],
                    )
                    (nc.sync if reg == 0 else nc.scalar).dma_start(dst, src)
                nc.vector.tensor_tensor(
                    Tbf[:, nh].rearrange("p (n t) -> p n t", n=2),
                    T[:, nh].rearrange("p (n t) -> p n t", n=2),
                    mkb,
                    op=mybir.AluOpType.mult,
                )
                for pk in range(2):
                    nc.tensor.matmul(
                        Y_ps[pk][:, nh * HF : (nh + 1) * HF],
                        Qbf[:, pk * 128 : (pk + 1) * 128],
                        Tbf[:, nh],
                        start=True,
                        stop=True,
                    )
                    nc.vector.tensor_scalar_add(
                        ysb[:, pk, nh],
                        Y_ps[pk][:, nh * HF : (nh + 1) * HF],
                        badap[:, pk : pk + 1],
                    )
                    (nc.sync if pk == 0 else nc.scalar).dma_start(
                        outr[pk][:, ns],
                        ysb[:, pk, nh].rearrange("p (n t) -> p n t", n=2),
                    )

    nc.compile()
    return nc


def kernel(x, Wk, bk, b_adap):
    global last_exec_time_ns
    if "nc" not in _cached:
        _cached["nc"] = build()
    nc = _cached["nc"]

    x = np.ascontiguousarray(x, dtype=np.float32)
    Wk = np.ascontiguousarray(Wk, dtype=np.float32)
    bk = np.ascontiguousarray(bk, dtype=np.float32)
    b_adap = np.ascontiguousarray(b_adap, dtype=np.float32)

    in_maps = [
        {"x": x[i * B : (i + 1) * B], "Wk": Wk, "bk": bk, "b_adap": b_adap}
        for i in range(NCORES)
    ]
    res = run_bass_kernel_spmd(
        nc,
        in_maps,
        core_ids=list(range(NCORES)),
        trace=bool(os.environ.get("KERNEL_TRACE")),
    )
    last_exec_time_ns = res.exec_time_ns
    out = np.concatenate(
        [res.results[i]["out"].reshape(B, P, H, W) for i in range(NCORES)], axis=0
    )
    return out
